# revision 4
# baseline (speedup 1.0000x reference)
"""Trainium2 Bass kernel for nn_AGSISpaBlock (pre-norm MHA + GELU FFN block).

Sharding: 8 cores; core c handles batch b = c//2 and query-half qh = c%2.
Each core receives its batch's tokens PERMUTED so its 2048 local query rows
come first (attention is permutation-invariant over keys, so one SPMD graph
serves all cores). No collectives needed.

Dataflow on each core (all matmuls bf16 with fp32 PSUM accumulation):
  LN1 (token-major, bn_stats) -> xn^T via PE transposes (feature-major)
  Q^T/K^T projections (feature-major), V token-major with importance folded
  in as exp(importance) row-scaling (V'' = eimp * [V | 1]).
  Scores computed keys-major: S^T[k, q] chunks via head-pair row-packed
  matmuls (heads 2p/2p+1 occupy PE rows 0-63/64-127 concurrently).
  exp(S/8) on ScalarE straight out of PSUM (3-bank groups -> one ACT op).
  ctx^T[hd+1, q] accumulated over key chunks (ones column of V'' gives the
  softmax denominator L). Normalize by 1/L after transposing ctx to
  token-major (L becomes a per-partition column). Transpose back, Wo,
  residual, LN2, FFN (exact-erf Gelu on ScalarE), final residual.
"""

import sys

if "/opt/trn_rl_repo" not in sys.path:
    sys.path.insert(0, "/opt/trn_rl_repo")

import numpy as np
import ml_dtypes

import concourse.bass as bass
import concourse.tile as tile
from concourse import bacc, mybir
from concourse.bass_utils import run_bass_kernel_spmd

F32 = mybir.dt.float32
BF16 = mybir.dt.bfloat16
BF = ml_dtypes.bfloat16

B, N, D = 4, 4096, 256
H, HD = 4, 64
FF = 512
EPS = 1e-5
NQ = N // 2          # local queries per core
KC = N // 128        # key chunks (32)
QB = NQ // 512       # 512-wide query blocks (4)
QT = NQ // 128       # 128-wide query tiles (16)

_compiled = None


def _build():
    nc = bacc.Bacc("TRN2", target_bir_lowering=False, debug=False, num_devices=8)

    tok = nc.declare_dram_parameter("tokens", [N, D], F32, isOutput=False)
    imp = nc.declare_dram_parameter("imp", [128, KC], F32, isOutput=False)
    wq = nc.declare_dram_parameter("wq", [D, D], BF16, isOutput=False)
    wk = nc.declare_dram_parameter("wk", [D, D], BF16, isOutput=False)
    wv = nc.declare_dram_parameter("wv", [D, D], BF16, isOutput=False)
    wo = nc.declare_dram_parameter("wo", [D, D], BF16, isOutput=False)
    w1 = nc.declare_dram_parameter("w1", [D, FF], BF16, isOutput=False)
    w2 = nc.declare_dram_parameter("w2", [FF, D], BF16, isOutput=False)
    bq = nc.declare_dram_parameter("bq", [128, 2], F32, isOutput=False)
    bk = nc.declare_dram_parameter("bk", [128, 2], F32, isOutput=False)
    bvb = nc.declare_dram_parameter("bvb", [128, D], F32, isOutput=False)
    bo = nc.declare_dram_parameter("bo", [128, 2], F32, isOutput=False)
    b1 = nc.declare_dram_parameter("b1", [128, 4], F32, isOutput=False)
    b2 = nc.declare_dram_parameter("b2", [128, 2], F32, isOutput=False)
    idb = nc.declare_dram_parameter("idb", [128, 128], BF16, isOutputFalse := False)
    idf = nc.declare_dram_parameter("idf", [128, 128], F32, isOutput=False)
    out = nc.declare_dram_parameter("out", [NQ, D], F32, isOutput=True)

    EXP = mybir.ActivationFunctionType.Exp
    GELU = mybir.ActivationFunctionType.Gelu
    SQRT = mybir.ActivationFunctionType.Sqrt
    SUB = mybir.AluOpType.subtract
    MUL = mybir.AluOpType.mult
    ADD = mybir.AluOpType.add

    with tile.TileContext(nc) as tc:
        with (
            tc.tile_pool(name="singles", bufs=1) as S,
            tc.tile_pool(name="work", bufs=4) as W4,
            tc.tile_pool(name="stats", bufs=4) as ST,
        ):
            # ---- persistent SBUF tensors ----
            xnT = [S.tile([128, N], BF16, tag=f"xnT{c}", name=f"xnT{c}") for c in range(2)]
            qT = [S.tile([128, NQ], BF16, tag=f"qT{c}", name=f"qT{c}") for c in range(2)]
            kT = [S.tile([128, N], BF16, tag=f"kT{c}", name=f"kT{c}") for c in range(2)]
            v2 = S.tile([128, KC, H * (HD + 1)], BF16, tag="v2", name="v2")
            ctx_sb = [S.tile([65, NQ], F32, tag=f"ctx{h}", name=f"ctx{h}") for h in range(H)]
            ctxnT = [S.tile([128, NQ], BF16, tag=f"ctxnT{c}", name=f"ctxnT{c}") for c in range(2)]
            aoT = [S.tile([128, NQ], BF16, tag=f"aoT{c}", name=f"aoT{c}") for c in range(2)]
            xtok = S.tile([128, QT, D], F32, tag="xtok", name="xtok")
            xn2T = [S.tile([128, NQ], BF16, tag=f"xn2T{c}", name=f"xn2T{c}") for c in range(2)]
            hT = [S.tile([128, NQ], BF16, tag=f"hT{f}", name=f"hT{f}") for f in range(4)]
            yT = [S.tile([128, NQ], BF16, tag=f"yT{c}", name=f"yT{c}") for c in range(2)]

            # ---- weights / consts ----
            wq_sb = [S.tile([128, D], BF16, tag=f"wq{c}", name=f"wq{c}") for c in range(2)]
            wk_sb = [S.tile([128, D], BF16, tag=f"wk{c}", name=f"wk{c}") for c in range(2)]
            wv_sb = [S.tile([128, D], BF16, tag=f"wv{c}", name=f"wv{c}") for c in range(2)]
            wo_sb = [S.tile([128, D], BF16, tag=f"wo{c}", name=f"wo{c}") for c in range(2)]
            w1_sb = [S.tile([128, FF], BF16, tag=f"w1{c}", name=f"w1{c}") for c in range(2)]
            w2_sb = [S.tile([128, D], BF16, tag=f"w2{c}", name=f"w2{c}") for c in range(4)]
            for c in range(2):
                nc.sync.dma_start(out=wq_sb[c][:], in_=wq[128 * c:128 * (c + 1), :])
                nc.sync.dma_start(out=wk_sb[c][:], in_=wk[128 * c:128 * (c + 1), :])
                nc.sync.dma_start(out=wv_sb[c][:], in_=wv[128 * c:128 * (c + 1), :])
                nc.sync.dma_start(out=wo_sb[c][:], in_=wo[128 * c:128 * (c + 1), :])
                nc.sync.dma_start(out=w1_sb[c][:], in_=w1[128 * c:128 * (c + 1), :])
            for c in range(4):
                nc.sync.dma_start(out=w2_sb[c][:], in_=w2[128 * c:128 * (c + 1), :])
            bq_sb = S.tile([128, 2], F32, tag="bq", name="bq")
            bk_sb = S.tile([128, 2], F32, tag="bk", name="bk")
            bvb_sb = S.tile([128, D], F32, tag="bvb", name="bvb")
            bo_sb = S.tile([128, 2], F32, tag="bo", name="bo")
            b1_sb = S.tile([128, 4], F32, tag="b1", name="b1")
            b2_sb = S.tile([128, 2], F32, tag="b2", name="b2")
            nc.sync.dma_start(out=bq_sb[:], in_=bq[:])
            nc.sync.dma_start(out=bk_sb[:], in_=bk[:])
            nc.sync.dma_start(out=bvb_sb[:], in_=bvb[:])
            nc.sync.dma_start(out=bo_sb[:], in_=bo[:])
            nc.sync.dma_start(out=b1_sb[:], in_=b1[:])
            nc.sync.dma_start(out=b2_sb[:], in_=b2[:])
            idb_sb = S.tile([128, 128], BF16, tag="idb", name="idb")
            idf_sb = S.tile([128, 128], F32, tag="idf", name="idf")
            nc.sync.dma_start(out=idb_sb[:], in_=idb[:])
            nc.sync.dma_start(out=idf_sb[:], in_=idf[:])
            imp_sb = S.tile([128, KC], F32, tag="imp", name="imp")
            nc.sync.dma_start(out=imp_sb[:], in_=imp[:])
            eimp_sb = S.tile([128, KC], F32, tag="eimp", name="eimp")
            nc.scalar.activation(out=eimp_sb[:], in_=imp_sb[:], func=EXP)
            eps_sb = S.tile([128, 1], F32, tag="eps", name="eps")
            nc.vector.memset(eps_sb[:], EPS)

            # ================= P1: LN1 + transpose to xn^T =================
            with tc.tile_pool(name="p1ps", bufs=4, space="PSUM") as P1:
                for i in range(N // 128):
                    tt = W4.tile([128, D], F32, tag="tok", name="tok")
                    nc.sync.dma_start(out=tt[:], in_=tok[128 * i:128 * (i + 1), :])
                    st = ST.tile([128, 6], F32, tag="st", name="st")
                    nc.vector.bn_stats(out=st[:], in_=tt[:])
                    mv = ST.tile([128, 2], F32, tag="mv", name="mv")
                    nc.vector.bn_aggr(out=mv[:], in_=st[:])
                    sd = ST.tile([128, 1], F32, tag="sd", name="sd")
                    nc.scalar.activation(out=sd[:], in_=mv[:, 1:2], func=SQRT,
                                         bias=eps_sb[:], scale=1.0)
                    rs = ST.tile([128, 1], F32, tag="rs", name="rs")
                    nc.vector.reciprocal(out=rs[:], in_=sd[:])
                    xb = W4.tile([128, D], BF16, tag="xnb", name="xnb")
                    nc.vector.tensor_scalar(out=xb[:], in0=tt[:], scalar1=mv[:, 0:1],
                                            scalar2=rs[:], op0=SUB, op1=MUL)
                    tp = P1.tile([128, 2, 128], BF16, tag="p1t", name="p1t")
                    nc.tensor.transpose(tp[:, 0, :], xb[:, 0:128], idb_sb[:])
                    nc.tensor.transpose(tp[:, 1, :], xb[:, 128:256], idb_sb[:])
                    eng = nc.vector if i % 2 == 0 else nc.gpsimd
                    nc.vector.tensor_copy(out=xnT[0][:, 128 * i:128 * (i + 1)], in_=tp[:, 0, :])
                    nc.scalar.copy(out=xnT[1][:, 128 * i:128 * (i + 1)], in_=tp[:, 1, :])

            # ================= P2: Q/K/V projections =================
            with tc.tile_pool(name="p2ps", bufs=4, space="PSUM") as P2:
                for m in range(2):  # dout chunk
                    for q in range(QB):
                        ps = P2.tile([128, 512], F32, tag="qk", name="qk")
                        for c in range(2):
                            nc.tensor.matmul(ps[:], wq_sb[c][:, 128 * m:128 * (m + 1)],
                                             xnT[c][:, 512 * q:512 * (q + 1)],
                                             start=(c == 0), stop=(c == 1))
                        nc.vector.tensor_scalar(out=qT[m][:, 512 * q:512 * (q + 1)], in0=ps[:],
                                                scalar1=bq_sb[:, m:m + 1], scalar2=None, op0=ADD)
                    for q in range(N // 512):
                        ps = P2.tile([128, 512], F32, tag="qk", name="qk")
                        for c in range(2):
                            nc.tensor.matmul(ps[:], wk_sb[c][:, 128 * m:128 * (m + 1)],
                                             xnT[c][:, 512 * q:512 * (q + 1)],
                                             start=(c == 0), stop=(c == 1))
                        nc.vector.tensor_scalar(out=kT[m][:, 512 * q:512 * (q + 1)], in0=ps[:],
                                                scalar1=bk_sb[:, m:m + 1], scalar2=None, op0=ADD)
                for kc in range(KC):
                    ps = P2.tile([128, D], F32, tag="v", name="v")
                    for c in range(2):
                        nc.tensor.matmul(ps[:], xnT[c][:, 128 * kc:128 * (kc + 1)],
                                         wv_sb[c][:], start=(c == 0), stop=(c == 1))
                    vt = W4.tile([128, D], BF16, tag="vtmp", name="vtmp")
                    nc.vector.tensor_tensor(out=vt[:], in0=ps[:], in1=bvb_sb[:], op=ADD)
                    for h in range(H):
                        nc.gpsimd.tensor_scalar(out=v2[:, kc, 65 * h:65 * h + 64],
                                                in0=vt[:, 64 * h:64 * (h + 1)],
                                                scalar1=eimp_sb[:, kc:kc + 1], scalar2=None, op0=MUL)
                        nc.gpsimd.tensor_copy(out=v2[:, kc, 65 * h + 64:65 * h + 65],
                                              in_=eimp_sb[:, kc:kc + 1])

            # ================= P3: attention =================
            with (
                tc.tile_pool(name="p3s", bufs=2, space="PSUM") as P3S,
                tc.tile_pool(name="p3c", bufs=1, space="PSUM") as P3C,
            ):
                for p in range(2):  # head pair
                    for q in range(QB):
                        cps = [P3C.tile([65, 512], F32, tag=f"ctxps{hp}", name=f"ctxps{hp}") for hp in range(2)]
                        # slots: (kc, hp) pairs in order; groups of 3 psum banks
                        slots = [(kc, hp) for kc in range(KC) for hp in range(2)]
                        g = 0
                        while g < len(slots):
                            n = min(3, len(slots) - g)
                            sg = P3S.tile([128, 3, 512], F32, tag="sgrp", name="sgrp")
                            for j in range(n):
                                kc, hp = slots[g + j]
                                nc.tensor.matmul(
                                    sg[:, j, :],
                                    kT[p][64 * hp:64 * (hp + 1), 128 * kc:128 * (kc + 1)],
                                    qT[p][64 * hp:64 * (hp + 1), 512 * q:512 * (q + 1)],
                                    start=True, stop=True, skip_group_check=True)
                            pt = W4.tile([128, 3, 512], BF16, tag="ptile", name="ptile")
                            nc.scalar.activation(out=pt[:, 0:n, :], in_=sg[:, 0:n, :],
                                                 func=EXP, scale=0.125)
                            for j in range(n):
                                kc, hp = slots[g + j]
                                h = 2 * p + hp
                                nc.tensor.matmul(cps[hp][:], v2[:, kc, 65 * h:65 * (h + 1)],
                                                 pt[:, j, :], start=(kc == 0), stop=(kc == KC - 1),
                                                 skip_group_check=True)
                            g += n
                        for hp in range(2):
                            nc.vector.tensor_copy(out=ctx_sb[2 * p + hp][:, 512 * q:512 * (q + 1)],
                                                  in_=cps[hp][:])

            # ============ P4/P5: normalize by 1/L, back to feature-major ============
            with tc.tile_pool(name="p45", bufs=4, space="PSUM") as P45:
                for t in range(QT):
                    ck = W4.tile([128, D], BF16, tag="ctok", name="ctok")
                    for h in range(H):
                        tp = P45.tile([128, 65], F32, tag="ctr", name="ctr")
                        nc.tensor.transpose(tp[:], ctx_sb[h][:, 128 * t:128 * (t + 1)],
                                            idf_sb[0:65, 0:65])
                        rl = ST.tile([128, 1], F32, tag="rl", name="rl")
                        nc.vector.reciprocal(out=rl[:], in_=tp[:, 64:65])
                        nc.vector.tensor_scalar(out=ck[:, 64 * h:64 * (h + 1)], in0=tp[:, 0:64],
                                                scalar1=rl[:], scalar2=None, op0=MUL)
                    tb = P45.tile([128, 2, 128], BF16, tag="ctb", name="ctb")
                    nc.tensor.transpose(tb[:, 0, :], ck[:, 0:128], idb_sb[:])
                    nc.tensor.transpose(tb[:, 1, :], ck[:, 128:256], idb_sb[:])
                    nc.vector.tensor_copy(out=ctxnT[0][:, 128 * t:128 * (t + 1)], in_=tb[:, 0, :])
                    nc.scalar.copy(out=ctxnT[1][:, 128 * t:128 * (t + 1)], in_=tb[:, 1, :])

            # ================= P6/P7: Wo + residual =================
            with tc.tile_pool(name="p67", bufs=4, space="PSUM") as P67:
                for m in range(2):
                    for q in range(QB):
                        ps = P67.tile([128, 512], F32, tag="ao", name="ao")
                        for c in range(2):
                            nc.tensor.matmul(ps[:], wo_sb[c][:, 128 * m:128 * (m + 1)],
                                             ctxnT[c][:, 512 * q:512 * (q + 1)],
                                             start=(c == 0), stop=(c == 1))
                        nc.vector.tensor_scalar(out=aoT[m][:, 512 * q:512 * (q + 1)], in0=ps[:],
                                                scalar1=bo_sb[:, m:m + 1], scalar2=None, op0=ADD)
                for t in range(QT):
                    tb = P67.tile([128, 2, 128], BF16, tag="aot", name="aot")
                    nc.tensor.transpose(tb[:, 0, :], aoT[0][:, 128 * t:128 * (t + 1)], idb_sb[:])
                    nc.tensor.transpose(tb[:, 1, :], aoT[1][:, 128 * t:128 * (t + 1)], idb_sb[:])
                    tt = W4.tile([128, D], F32, tag="tok", name="tok")
                    nc.sync.dma_start(out=tt[:], in_=tok[128 * t:128 * (t + 1), :])
                    nc.vector.tensor_tensor(out=xtok[:, t, :], in0=tb.rearrange("p a b -> p (a b)"),
                                            in1=tt[:], op=ADD)

            # ================= P8: LN2 -> xn2^T =================
            with tc.tile_pool(name="p8ps", bufs=4, space="PSUM") as P8:
                for t in range(QT):
                    st = ST.tile([128, 6], F32, tag="st", name="st")
                    nc.vector.bn_stats(out=st[:], in_=xtok[:, t, :])
                    mv = ST.tile([128, 2], F32, tag="mv", name="mv")
                    nc.vector.bn_aggr(out=mv[:], in_=st[:])
                    sd = ST.tile([128, 1], F32, tag="sd", name="sd")
                    nc.scalar.activation(out=sd[:], in_=mv[:, 1:2], func=SQRT,
                                         bias=eps_sb[:], scale=1.0)
                    rs = ST.tile([128, 1], F32, tag="rs", name="rs")
                    nc.vector.reciprocal(out=rs[:], in_=sd[:])
                    xb = W4.tile([128, D], BF16, tag="xnb", name="xnb")
                    nc.vector.tensor_scalar(out=xb[:], in0=xtok[:, t, :], scalar1=mv[:, 0:1],
                                            scalar2=rs[:], op0=SUB, op1=MUL)
                    tp = P8.tile([128, 2, 128], BF16, tag="p8t", name="p8t")
                    nc.tensor.transpose(tp[:, 0, :], xb[:, 0:128], idb_sb[:])
                    nc.tensor.transpose(tp[:, 1, :], xb[:, 128:256], idb_sb[:])
                    nc.vector.tensor_copy(out=xn2T[0][:, 128 * t:128 * (t + 1)], in_=tp[:, 0, :])
                    nc.scalar.copy(out=xn2T[1][:, 128 * t:128 * (t + 1)], in_=tp[:, 1, :])

            # ================= P9: FFN1 + gelu =================
            with tc.tile_pool(name="p9ps", bufs=2, space="PSUM") as P9:
                for f in range(4):
                    ps = P9.tile([128, QB, 512], F32, tag="ffg", name="ffg")
                    for q in range(QB):
                        for c in range(2):
                            nc.tensor.matmul(ps[:, q, :], w1_sb[c][:, 128 * f:128 * (f + 1)],
                                             xn2T[c][:, 512 * q:512 * (q + 1)],
                                             start=(c == 0), stop=(c == 1), skip_group_check=True)
                    nc.scalar.activation(out=hT[f].rearrange("p (a b) -> p a b", a=QB),
                                         in_=ps[:], func=GELU, bias=b1_sb[:, f:f + 1], scale=1.0)

            # ================= P10: FFN2 + final residual =================
            with tc.tile_pool(name="pAps", bufs=2, space="PSUM") as PA:
                for m in range(2):
                    for q in range(QB):
                        ps = PA.tile([128, 512], F32, tag="y", name="y")
                        for c in range(4):
                            nc.tensor.matmul(ps[:], w2_sb[c][:, 128 * m:128 * (m + 1)],
                                             hT[c][:, 512 * q:512 * (q + 1)],
                                             start=(c == 0), stop=(c == 3))
                        nc.vector.tensor_scalar(out=yT[m][:, 512 * q:512 * (q + 1)], in0=ps[:],
                                                scalar1=b2_sb[:, m:m + 1], scalar2=None, op0=ADD)
                for t in range(QT):
                    tb = PA.tile([128, 2, 128], BF16, tag="yt", name="yt")
                    nc.tensor.transpose(tb[:, 0, :], yT[0][:, 128 * t:128 * (t + 1)], idb_sb[:])
                    nc.tensor.transpose(tb[:, 1, :], yT[1][:, 128 * t:128 * (t + 1)], idb_sb[:])
                    ot = W4.tile([128, D], F32, tag="ot", name="ot")
                    nc.vector.tensor_tensor(out=ot[:], in0=tb.rearrange("p a b -> p (a b)"),
                                            in1=xtok[:, t, :], op=ADD)
                    nc.sync.dma_start(out=out[128 * t:128 * (t + 1), :], in_=ot[:])

    nc.compile()
    return nc


def _get_compiled():
    global _compiled
    if _compiled is None:
        _compiled = _build()
    return _compiled


def _prep_in_maps(tokens, importance, norm1_w, norm1_b, Wq, bq, Wk, bk, Wv, bv,
                  Wo, bo, norm2_w, norm2_b, W1, b1, W2, b2):
    f32 = np.float32
    tokens = np.asarray(tokens, f32)
    importance = np.asarray(importance, f32)

    # fold LN affine params into the following projection weights
    Wq_f = (np.asarray(norm1_w, f32)[:, None] * np.asarray(Wq, f32))
    Wk_f = (np.asarray(norm1_w, f32)[:, None] * np.asarray(Wk, f32))
    Wv_f = (np.asarray(norm1_w, f32)[:, None] * np.asarray(Wv, f32))
    bq_f = np.asarray(norm1_b, f32) @ np.asarray(Wq, f32) + np.asarray(bq, f32)
    bk_f = np.asarray(norm1_b, f32) @ np.asarray(Wk, f32) + np.asarray(bk, f32)
    bv_f = np.asarray(norm1_b, f32) @ np.asarray(Wv, f32) + np.asarray(bv, f32)
    W1_f = (np.asarray(norm2_w, f32)[:, None] * np.asarray(W1, f32))
    b1_f = np.asarray(norm2_b, f32) @ np.asarray(W1, f32) + np.asarray(b1, f32)

    common = {
        "wq": Wq_f.astype(BF), "wk": Wk_f.astype(BF), "wv": Wv_f.astype(BF),
        "wo": np.asarray(Wo, f32).astype(BF),
        "w1": W1_f.astype(BF), "w2": np.asarray(W2, f32).astype(BF),
        "bq": np.ascontiguousarray(bq_f.reshape(2, 128).T.astype(f32)),
        "bk": np.ascontiguousarray(bk_f.reshape(2, 128).T.astype(f32)),
        "bvb": np.ascontiguousarray(np.broadcast_to(bv_f, (128, D)).astype(f32)),
        "bo": np.ascontiguousarray(np.asarray(bo, f32).reshape(2, 128).T),
        "b1": np.ascontiguousarray(b1_f.reshape(4, 128).T.astype(f32)),
        "b2": np.ascontiguousarray(np.asarray(b2, f32).reshape(2, 128).T),
        "idb": np.eye(128, dtype=f32).astype(BF),
        "idf": np.eye(128, dtype=f32),
    }

    in_maps = []
    for c in range(8):
        b = c // 2
        qh = c % 2
        qs = qh * NQ
        perm = np.r_[qs:qs + NQ, (0 if qh else NQ):(NQ if qh else N)]
        toks = np.ascontiguousarray(tokens[b][perm])
        impp = np.ascontiguousarray(importance[b][perm].reshape(KC, 128).T.astype(f32))
        in_maps.append({"tokens": toks, "imp": impp, **common})
    return in_maps


def _run(in_maps, trace=False):
    nc = _get_compiled()
    return run_bass_kernel_spmd(nc, in_maps, core_ids=list(range(8)), trace=trace)


def kernel(**inputs) -> np.ndarray:
    in_maps = _prep_in_maps(**inputs)
    res = _run(in_maps, trace=False)
    out = np.empty((B, N, D), np.float32)
    for c in range(8):
        b = c // 2
        qs = (c % 2) * NQ
        out[b, qs:qs + NQ] = res.results[c]["out"]
    return out


def kernel_traced(**inputs):
    """Like kernel() but with NTFF profiling; returns (out, exec_time_ns)."""
    in_maps = _prep_in_maps(**inputs)
    res = _run(in_maps, trace=True)
    out = np.empty((B, N, D), np.float32)
    for c in range(8):
        b = c // 2
        qs = (c % 2) * NQ
        out[b, qs:qs + NQ] = res.results[c]["out"]
    return out, res.exec_time_ns


# revision 5
# speedup vs baseline: 1.2525x; 1.2525x over previous
"""Trainium2 Bass kernel for nn_AGSISpaBlock (pre-norm MHA + GELU FFN block).

Sharding: 8 cores; core c handles batch b = c//2 and query-half qh = c%2.
Each core receives its batch's tokens PERMUTED so its 2048 local query rows
come first (attention is permutation-invariant over keys, so one SPMD graph
serves all cores). No collectives needed.

Dataflow on each core (all matmuls bf16 with fp32 PSUM accumulation):
  LN1 (token-major, bn_stats) -> xn^T via PE transposes (feature-major)
  Q^T/K^T projections (feature-major), V token-major with importance folded
  in as exp(importance) row-scaling (V'' = eimp * [V | 1]).
  Scores computed keys-major: S^T[k, q] chunks via head-pair row-packed
  matmuls (heads 2p/2p+1 occupy PE rows 0-63/64-127 concurrently).
  exp(S/8) on ScalarE straight out of PSUM (3-bank groups -> one ACT op).
  ctx^T[hd+1, q] accumulated over key chunks (ones column of V'' gives the
  softmax denominator L). Normalize by 1/L after transposing ctx to
  token-major (L becomes a per-partition column). Transpose back, Wo,
  residual, LN2, FFN (exact-erf Gelu on ScalarE), final residual.
"""

import sys

if "/opt/trn_rl_repo" not in sys.path:
    sys.path.insert(0, "/opt/trn_rl_repo")

import numpy as np
import ml_dtypes

import concourse.bass as bass
import concourse.tile as tile
from concourse import bacc, mybir
from concourse.bass_utils import run_bass_kernel_spmd

F32 = mybir.dt.float32
BF16 = mybir.dt.bfloat16
BF = ml_dtypes.bfloat16

B, N, D = 4, 4096, 256
H, HD = 4, 64
FF = 512
EPS = 1e-5
NQ = N // 2          # local queries per core
KC = N // 128        # key chunks (32)
QB = NQ // 512       # 512-wide query blocks (4)
QT = NQ // 128       # 128-wide query tiles (16)

_compiled = None


def _build():
    nc = bacc.Bacc("TRN2", target_bir_lowering=False, debug=False, num_devices=8)

    tok = nc.declare_dram_parameter("tokens", [N, D], F32, isOutput=False)
    imp = nc.declare_dram_parameter("imp", [128, KC], F32, isOutput=False)
    wq = nc.declare_dram_parameter("wq", [D, D], BF16, isOutput=False)
    wk = nc.declare_dram_parameter("wk", [D, D], BF16, isOutput=False)
    wv = nc.declare_dram_parameter("wv", [D, D], BF16, isOutput=False)
    wo = nc.declare_dram_parameter("wo", [D, D], BF16, isOutput=False)
    w1 = nc.declare_dram_parameter("w1", [D, FF], BF16, isOutput=False)
    w2 = nc.declare_dram_parameter("w2", [FF, D], BF16, isOutput=False)
    bq = nc.declare_dram_parameter("bq", [128, 2], F32, isOutput=False)
    bk = nc.declare_dram_parameter("bk", [128, 2], F32, isOutput=False)
    bvb = nc.declare_dram_parameter("bvb", [128, D], F32, isOutput=False)
    bo = nc.declare_dram_parameter("bo", [128, 2], F32, isOutput=False)
    b1 = nc.declare_dram_parameter("b1", [128, 4], F32, isOutput=False)
    b2 = nc.declare_dram_parameter("b2", [128, 2], F32, isOutput=False)
    idb = nc.declare_dram_parameter("idb", [128, 128], BF16, isOutputFalse := False)
    idf = nc.declare_dram_parameter("idf", [128, 128], F32, isOutput=False)
    out = nc.declare_dram_parameter("out", [NQ, D], F32, isOutput=True)

    EXP = mybir.ActivationFunctionType.Exp
    GELU = mybir.ActivationFunctionType.Gelu
    SQRT = mybir.ActivationFunctionType.Sqrt
    SUB = mybir.AluOpType.subtract
    MUL = mybir.AluOpType.mult
    ADD = mybir.AluOpType.add

    with tile.TileContext(nc) as tc:
        with (
            tc.tile_pool(name="singles", bufs=1) as S,
            tc.tile_pool(name="work", bufs=4) as W4,
            tc.tile_pool(name="stats", bufs=4) as ST,
        ):
            # ---- persistent SBUF tensors ----
            xnT = [S.tile([128, N], BF16, tag=f"xnT{c}", name=f"xnT{c}") for c in range(2)]
            qT = [S.tile([128, NQ], BF16, tag=f"qT{c}", name=f"qT{c}") for c in range(2)]
            kT = [S.tile([128, N], BF16, tag=f"kT{c}", name=f"kT{c}") for c in range(2)]
            v2 = S.tile([128, KC, H * (HD + 1)], BF16, tag="v2", name="v2")
            ctx_sb = [S.tile([65, NQ], F32, tag=f"ctx{h}", name=f"ctx{h}") for h in range(H)]
            ctxnT = [S.tile([128, NQ], BF16, tag=f"ctxnT{c}", name=f"ctxnT{c}") for c in range(2)]
            aoT = [S.tile([128, NQ], BF16, tag=f"aoT{c}", name=f"aoT{c}") for c in range(2)]
            xtok = S.tile([128, QT, D], F32, tag="xtok", name="xtok")
            xn2T = [S.tile([128, NQ], BF16, tag=f"xn2T{c}", name=f"xn2T{c}") for c in range(2)]
            hT = [S.tile([128, NQ], BF16, tag=f"hT{f}", name=f"hT{f}") for f in range(4)]
            yT = [S.tile([128, NQ], BF16, tag=f"yT{c}", name=f"yT{c}") for c in range(2)]

            # ---- weights / consts ----
            wq_sb = [S.tile([128, D], BF16, tag=f"wq{c}", name=f"wq{c}") for c in range(2)]
            wk_sb = [S.tile([128, D], BF16, tag=f"wk{c}", name=f"wk{c}") for c in range(2)]
            wv_sb = [S.tile([128, D], BF16, tag=f"wv{c}", name=f"wv{c}") for c in range(2)]
            wo_sb = [S.tile([128, D], BF16, tag=f"wo{c}", name=f"wo{c}") for c in range(2)]
            w1_sb = [S.tile([128, FF], BF16, tag=f"w1{c}", name=f"w1{c}") for c in range(2)]
            w2_sb = [S.tile([128, D], BF16, tag=f"w2{c}", name=f"w2{c}") for c in range(4)]
            for c in range(2):
                nc.sync.dma_start(out=wq_sb[c][:], in_=wq[128 * c:128 * (c + 1), :])
                nc.sync.dma_start(out=wk_sb[c][:], in_=wk[128 * c:128 * (c + 1), :])
                nc.sync.dma_start(out=wv_sb[c][:], in_=wv[128 * c:128 * (c + 1), :])
                nc.sync.dma_start(out=wo_sb[c][:], in_=wo[128 * c:128 * (c + 1), :])
                nc.sync.dma_start(out=w1_sb[c][:], in_=w1[128 * c:128 * (c + 1), :])
            for c in range(4):
                nc.sync.dma_start(out=w2_sb[c][:], in_=w2[128 * c:128 * (c + 1), :])
            bq_sb = S.tile([128, 2], F32, tag="bq", name="bq")
            bk_sb = S.tile([128, 2], F32, tag="bk", name="bk")
            bvb_sb = S.tile([128, D], F32, tag="bvb", name="bvb")
            bo_sb = S.tile([128, 2], F32, tag="bo", name="bo")
            b1_sb = S.tile([128, 4], F32, tag="b1", name="b1")
            b2_sb = S.tile([128, 2], F32, tag="b2", name="b2")
            nc.sync.dma_start(out=bq_sb[:], in_=bq[:])
            nc.sync.dma_start(out=bk_sb[:], in_=bk[:])
            nc.sync.dma_start(out=bvb_sb[:], in_=bvb[:])
            nc.sync.dma_start(out=bo_sb[:], in_=bo[:])
            nc.sync.dma_start(out=b1_sb[:], in_=b1[:])
            nc.sync.dma_start(out=b2_sb[:], in_=b2[:])
            idb_sb = S.tile([128, 128], BF16, tag="idb", name="idb")
            idf_sb = S.tile([128, 128], F32, tag="idf", name="idf")
            nc.sync.dma_start(out=idb_sb[:], in_=idb[:])
            nc.sync.dma_start(out=idf_sb[:], in_=idf[:])
            imp_sb = S.tile([128, KC], F32, tag="imp", name="imp")
            nc.sync.dma_start(out=imp_sb[:], in_=imp[:])
            eimp_sb = S.tile([128, KC], F32, tag="eimp", name="eimp")
            nc.scalar.activation(out=eimp_sb[:], in_=imp_sb[:], func=EXP)
            eps_sb = S.tile([128, 1], F32, tag="eps", name="eps")
            nc.vector.memset(eps_sb[:], EPS)

            # ================= P1: LN1 + transpose to xn^T =================
            with tc.tile_pool(name="p1ps", bufs=4, space="PSUM") as P1:
                for i in range(N // 128):
                    tt = W4.tile([128, D], F32, tag="tok", name="tok")
                    nc.sync.dma_start(out=tt[:], in_=tok[128 * i:128 * (i + 1), :])
                    st = ST.tile([128, 6], F32, tag="st", name="st")
                    nc.vector.bn_stats(out=st[:], in_=tt[:])
                    mv = ST.tile([128, 2], F32, tag="mv", name="mv")
                    nc.vector.bn_aggr(out=mv[:], in_=st[:])
                    sd = ST.tile([128, 1], F32, tag="sd", name="sd")
                    nc.scalar.activation(out=sd[:], in_=mv[:, 1:2], func=SQRT,
                                         bias=eps_sb[:], scale=1.0)
                    rs = ST.tile([128, 1], F32, tag="rs", name="rs")
                    nc.vector.reciprocal(out=rs[:], in_=sd[:])
                    xb = W4.tile([128, D], BF16, tag="xnb", name="xnb")
                    nc.vector.tensor_scalar(out=xb[:], in0=tt[:], scalar1=mv[:, 0:1],
                                            scalar2=rs[:], op0=SUB, op1=MUL)
                    tp = P1.tile([128, 2, 128], BF16, tag="p1t", name="p1t")
                    nc.tensor.transpose(tp[:, 0, :], xb[:, 0:128], idb_sb[:])
                    nc.tensor.transpose(tp[:, 1, :], xb[:, 128:256], idb_sb[:])
                    nc.vector.tensor_copy(out=xnT[0][:, 128 * i:128 * (i + 1)], in_=tp[:, 0, :])
                    nc.scalar.copy(out=xnT[1][:, 128 * i:128 * (i + 1)], in_=tp[:, 1, :])

            # ================= P2: Q/K/V projections =================
            with tc.tile_pool(name="p2ps", bufs=4, space="PSUM") as P2:
                for m in range(2):  # dout chunk
                    for q in range(QB):
                        ps = P2.tile([128, 512], F32, tag="qk", name="qk")
                        for c in range(2):
                            nc.tensor.matmul(ps[:], wq_sb[c][:, 128 * m:128 * (m + 1)],
                                             xnT[c][:, 512 * q:512 * (q + 1)],
                                             start=(c == 0), stop=(c == 1))
                        nc.vector.tensor_scalar(out=qT[m][:, 512 * q:512 * (q + 1)], in0=ps[:],
                                                scalar1=bq_sb[:, m:m + 1], scalar2=None, op0=ADD)
                    for q in range(N // 512):
                        ps = P2.tile([128, 512], F32, tag="qk", name="qk")
                        for c in range(2):
                            nc.tensor.matmul(ps[:], wk_sb[c][:, 128 * m:128 * (m + 1)],
                                             xnT[c][:, 512 * q:512 * (q + 1)],
                                             start=(c == 0), stop=(c == 1))
                        nc.vector.tensor_scalar(out=kT[m][:, 512 * q:512 * (q + 1)], in0=ps[:],
                                                scalar1=bk_sb[:, m:m + 1], scalar2=None, op0=ADD)
                for kc in range(KC):
                    ps = P2.tile([128, D], F32, tag="v", name="v")
                    for c in range(2):
                        nc.tensor.matmul(ps[:], xnT[c][:, 128 * kc:128 * (kc + 1)],
                                         wv_sb[c][:], start=(c == 0), stop=(c == 1))
                    vt = W4.tile([128, D], BF16, tag="vtmp", name="vtmp")
                    nc.vector.tensor_tensor(out=vt[:], in0=ps[:], in1=bvb_sb[:], op=ADD)
                    v2kc = v2[:, kc, :].rearrange("p (h j) -> p h j", h=H)
                    nc.vector.tensor_scalar(out=v2kc[:, :, 0:64],
                                            in0=vt[:].rearrange("p (h j) -> p h j", h=H),
                                            scalar1=eimp_sb[:, kc:kc + 1], scalar2=None, op0=MUL)
                    e1 = eimp_sb[:, kc:kc + 1]
                    ebc = bass.AP(tensor=e1.tensor, offset=e1.offset,
                                  ap=[e1.ap[0], [0, H], [0, 1]])
                    nc.vector.tensor_copy(out=v2kc[:, :, 64:65], in_=ebc)

            # ================= P3: attention =================
            with (
                tc.tile_pool(name="p3s", bufs=2, space="PSUM") as P3S,
                tc.tile_pool(name="p3c", bufs=1, space="PSUM") as P3C,
            ):
                for p in range(2):  # head pair
                    for q in range(QB):
                        cps = [P3C.tile([65, 512], F32, tag=f"ctxps{hp}", name=f"ctxps{hp}") for hp in range(2)]
                        # slots: (kc, hp) pairs in order; groups of 3 psum banks
                        slots = [(kc, hp) for kc in range(KC) for hp in range(2)]
                        g = 0
                        while g < len(slots):
                            n = min(3, len(slots) - g)
                            sg = P3S.tile([128, 3, 512], F32, tag="sgrp", name="sgrp")
                            for j in range(n):
                                kc, hp = slots[g + j]
                                nc.tensor.matmul(
                                    sg[:, j, :],
                                    kT[p][64 * hp:64 * (hp + 1), 128 * kc:128 * (kc + 1)],
                                    qT[p][64 * hp:64 * (hp + 1), 512 * q:512 * (q + 1)],
                                    start=True, stop=True, skip_group_check=True)
                            pt = W4.tile([128, 3, 512], BF16, tag="ptile", name="ptile")
                            nc.scalar.activation(out=pt[:, 0:n, :], in_=sg[:, 0:n, :],
                                                 func=EXP, scale=0.125)
                            for j in range(n):
                                kc, hp = slots[g + j]
                                h = 2 * p + hp
                                nc.tensor.matmul(cps[hp][:], v2[:, kc, 65 * h:65 * (h + 1)],
                                                 pt[:, j, :], start=(kc == 0), stop=(kc == KC - 1),
                                                 skip_group_check=True)
                            g += n
                        for hp in range(2):
                            nc.vector.tensor_copy(out=ctx_sb[2 * p + hp][:, 512 * q:512 * (q + 1)],
                                                  in_=cps[hp][:])

            # ============ P4/P5: normalize by 1/L, back to feature-major ============
            with tc.tile_pool(name="p45", bufs=4, space="PSUM") as P45:
                for t in range(QT):
                    ck = W4.tile([128, D], BF16, tag="ctok", name="ctok")
                    for h in range(H):
                        tp = P45.tile([128, 65], F32, tag="ctr", name="ctr")
                        nc.tensor.transpose(tp[:], ctx_sb[h][:, 128 * t:128 * (t + 1)],
                                            idf_sb[0:65, 0:65])
                        rl = ST.tile([128, 1], F32, tag="rl", name="rl")
                        nc.vector.reciprocal(out=rl[:], in_=tp[:, 64:65])
                        nc.vector.tensor_scalar(out=ck[:, 64 * h:64 * (h + 1)], in0=tp[:, 0:64],
                                                scalar1=rl[:], scalar2=None, op0=MUL)
                    tb = P45.tile([128, 2, 128], BF16, tag="ctb", name="ctb")
                    nc.tensor.transpose(tb[:, 0, :], ck[:, 0:128], idb_sb[:])
                    nc.tensor.transpose(tb[:, 1, :], ck[:, 128:256], idb_sb[:])
                    nc.vector.tensor_copy(out=ctxnT[0][:, 128 * t:128 * (t + 1)], in_=tb[:, 0, :])
                    nc.scalar.copy(out=ctxnT[1][:, 128 * t:128 * (t + 1)], in_=tb[:, 1, :])

            # ================= P6/P7: Wo + residual =================
            with tc.tile_pool(name="p67", bufs=4, space="PSUM") as P67:
                for m in range(2):
                    for q in range(QB):
                        ps = P67.tile([128, 512], F32, tag="ao", name="ao")
                        for c in range(2):
                            nc.tensor.matmul(ps[:], wo_sb[c][:, 128 * m:128 * (m + 1)],
                                             ctxnT[c][:, 512 * q:512 * (q + 1)],
                                             start=(c == 0), stop=(c == 1))
                        nc.vector.tensor_scalar(out=aoT[m][:, 512 * q:512 * (q + 1)], in0=ps[:],
                                                scalar1=bo_sb[:, m:m + 1], scalar2=None, op0=ADD)
                for t in range(QT):
                    tb = P67.tile([128, 2, 128], BF16, tag="aot", name="aot")
                    nc.tensor.transpose(tb[:, 0, :], aoT[0][:, 128 * t:128 * (t + 1)], idb_sb[:])
                    nc.tensor.transpose(tb[:, 1, :], aoT[1][:, 128 * t:128 * (t + 1)], idb_sb[:])
                    tt = W4.tile([128, D], F32, tag="tok", name="tok")
                    nc.sync.dma_start(out=tt[:], in_=tok[128 * t:128 * (t + 1), :])
                    nc.vector.tensor_tensor(out=xtok[:, t, :], in0=tb.rearrange("p a b -> p (a b)"),
                                            in1=tt[:], op=ADD)

            # ================= P8: LN2 -> xn2^T =================
            with tc.tile_pool(name="p8ps", bufs=4, space="PSUM") as P8:
                for t in range(QT):
                    st = ST.tile([128, 6], F32, tag="st", name="st")
                    nc.vector.bn_stats(out=st[:], in_=xtok[:, t, :])
                    mv = ST.tile([128, 2], F32, tag="mv", name="mv")
                    nc.vector.bn_aggr(out=mv[:], in_=st[:])
                    sd = ST.tile([128, 1], F32, tag="sd", name="sd")
                    nc.scalar.activation(out=sd[:], in_=mv[:, 1:2], func=SQRT,
                                         bias=eps_sb[:], scale=1.0)
                    rs = ST.tile([128, 1], F32, tag="rs", name="rs")
                    nc.vector.reciprocal(out=rs[:], in_=sd[:])
                    xb = W4.tile([128, D], BF16, tag="xnb", name="xnb")
                    nc.vector.tensor_scalar(out=xb[:], in0=xtok[:, t, :], scalar1=mv[:, 0:1],
                                            scalar2=rs[:], op0=SUB, op1=MUL)
                    tp = P8.tile([128, 2, 128], BF16, tag="p8t", name="p8t")
                    nc.tensor.transpose(tp[:, 0, :], xb[:, 0:128], idb_sb[:])
                    nc.tensor.transpose(tp[:, 1, :], xb[:, 128:256], idb_sb[:])
                    nc.vector.tensor_copy(out=xn2T[0][:, 128 * t:128 * (t + 1)], in_=tp[:, 0, :])
                    nc.scalar.copy(out=xn2T[1][:, 128 * t:128 * (t + 1)], in_=tp[:, 1, :])

            # ================= P9: FFN1 + gelu =================
            with tc.tile_pool(name="p9ps", bufs=2, space="PSUM") as P9:
                for f in range(4):
                    ps = P9.tile([128, QB, 512], F32, tag="ffg", name="ffg")
                    for q in range(QB):
                        for c in range(2):
                            nc.tensor.matmul(ps[:, q, :], w1_sb[c][:, 128 * f:128 * (f + 1)],
                                             xn2T[c][:, 512 * q:512 * (q + 1)],
                                             start=(c == 0), stop=(c == 1), skip_group_check=True)
                    nc.scalar.activation(out=hT[f].rearrange("p (a b) -> p a b", a=QB),
                                         in_=ps[:], func=GELU, bias=b1_sb[:, f:f + 1], scale=1.0)

            # ================= P10: FFN2 + final residual =================
            with tc.tile_pool(name="pAps", bufs=2, space="PSUM") as PA:
                for m in range(2):
                    for q in range(QB):
                        ps = PA.tile([128, 512], F32, tag="y", name="y")
                        for c in range(4):
                            nc.tensor.matmul(ps[:], w2_sb[c][:, 128 * m:128 * (m + 1)],
                                             hT[c][:, 512 * q:512 * (q + 1)],
                                             start=(c == 0), stop=(c == 3))
                        nc.vector.tensor_scalar(out=yT[m][:, 512 * q:512 * (q + 1)], in0=ps[:],
                                                scalar1=b2_sb[:, m:m + 1], scalar2=None, op0=ADD)
                for t in range(QT):
                    tb = PA.tile([128, 2, 128], BF16, tag="yt", name="yt")
                    nc.tensor.transpose(tb[:, 0, :], yT[0][:, 128 * t:128 * (t + 1)], idb_sb[:])
                    nc.tensor.transpose(tb[:, 1, :], yT[1][:, 128 * t:128 * (t + 1)], idb_sb[:])
                    ot = W4.tile([128, D], F32, tag="ot", name="ot")
                    nc.vector.tensor_tensor(out=ot[:], in0=tb.rearrange("p a b -> p (a b)"),
                                            in1=xtok[:, t, :], op=ADD)
                    nc.sync.dma_start(out=out[128 * t:128 * (t + 1), :], in_=ot[:])

    nc.compile()
    return nc


def _get_compiled():
    global _compiled
    if _compiled is None:
        _compiled = _build()
    return _compiled


def _prep_in_maps(tokens, importance, norm1_w, norm1_b, Wq, bq, Wk, bk, Wv, bv,
                  Wo, bo, norm2_w, norm2_b, W1, b1, W2, b2):
    f32 = np.float32
    tokens = np.asarray(tokens, f32)
    importance = np.asarray(importance, f32)

    # fold LN affine params into the following projection weights
    Wq_f = (np.asarray(norm1_w, f32)[:, None] * np.asarray(Wq, f32))
    Wk_f = (np.asarray(norm1_w, f32)[:, None] * np.asarray(Wk, f32))
    Wv_f = (np.asarray(norm1_w, f32)[:, None] * np.asarray(Wv, f32))
    bq_f = np.asarray(norm1_b, f32) @ np.asarray(Wq, f32) + np.asarray(bq, f32)
    bk_f = np.asarray(norm1_b, f32) @ np.asarray(Wk, f32) + np.asarray(bk, f32)
    bv_f = np.asarray(norm1_b, f32) @ np.asarray(Wv, f32) + np.asarray(bv, f32)
    W1_f = (np.asarray(norm2_w, f32)[:, None] * np.asarray(W1, f32))
    b1_f = np.asarray(norm2_b, f32) @ np.asarray(W1, f32) + np.asarray(b1, f32)

    common = {
        "wq": Wq_f.astype(BF), "wk": Wk_f.astype(BF), "wv": Wv_f.astype(BF),
        "wo": np.asarray(Wo, f32).astype(BF),
        "w1": W1_f.astype(BF), "w2": np.asarray(W2, f32).astype(BF),
        "bq": np.ascontiguousarray(bq_f.reshape(2, 128).T.astype(f32)),
        "bk": np.ascontiguousarray(bk_f.reshape(2, 128).T.astype(f32)),
        "bvb": np.ascontiguousarray(np.broadcast_to(bv_f, (128, D)).astype(f32)),
        "bo": np.ascontiguousarray(np.asarray(bo, f32).reshape(2, 128).T),
        "b1": np.ascontiguousarray(b1_f.reshape(4, 128).T.astype(f32)),
        "b2": np.ascontiguousarray(np.asarray(b2, f32).reshape(2, 128).T),
        "idb": np.eye(128, dtype=f32).astype(BF),
        "idf": np.eye(128, dtype=f32),
    }

    in_maps = []
    for c in range(8):
        b = c // 2
        qh = c % 2
        qs = qh * NQ
        perm = np.r_[qs:qs + NQ, (0 if qh else NQ):(NQ if qh else N)]
        toks = np.ascontiguousarray(tokens[b][perm])
        impp = np.ascontiguousarray(importance[b][perm].reshape(KC, 128).T.astype(f32))
        in_maps.append({"tokens": toks, "imp": impp, **common})
    return in_maps


def _run(in_maps, trace=False):
    nc = _get_compiled()
    return run_bass_kernel_spmd(nc, in_maps, core_ids=list(range(8)), trace=trace)


def kernel(**inputs) -> np.ndarray:
    in_maps = _prep_in_maps(**inputs)
    res = _run(in_maps, trace=False)
    out = np.empty((B, N, D), np.float32)
    for c in range(8):
        b = c // 2
        qs = (c % 2) * NQ
        out[b, qs:qs + NQ] = res.results[c]["out"]
    return out


def kernel_traced(**inputs):
    """Like kernel() but with NTFF profiling; returns (out, exec_time_ns)."""
    in_maps = _prep_in_maps(**inputs)
    res = _run(in_maps, trace=True)
    out = np.empty((B, N, D), np.float32)
    for c in range(8):
        b = c // 2
        qs = (c % 2) * NQ
        out[b, qs:qs + NQ] = res.results[c]["out"]
    return out, res.exec_time_ns
